# revision 1
# baseline (speedup 1.0000x reference)
import numpy as np
import jax
import jax.numpy as jnp
from functools import partial

# Problem constants (nn_AdaTTSp): hardcoded per harness rules.
L, T, E, D, H = 2, 8, 2, 128, 128
NE = T * E  # 16
M = 8  # number of NeuronCores; data-parallel over batch


def _forward(x, w1, b1, w2, b2, gate_w, gate_b, sew):
    # x: [b, T, D] local shard
    for l in range(L):
        x_exp = jnp.repeat(x, E, axis=1)  # [b, NE, D]
        h = jax.nn.relu(jnp.einsum('bed,edh->beh', x_exp, w1[l]) + b1[l])
        eo = jax.nn.relu(jnp.einsum('beh,eho->beo', h, w2[l]) + b2[l])
        gates = jax.nn.softmax(
            jnp.einsum('btd,tde->bte', x, gate_w[l]) + gate_b[l], axis=-1)
        fused = jnp.einsum('bte,beh->bth', gates, eo)
        eo_r = eo.reshape(eo.shape[0], T, E, -1)
        fused = fused + jnp.einsum('te,bteh->bth', sew[l], eo_r)
        x = fused
    return x


_pfwd = jax.pmap(_forward, axis_name='x',
                 in_axes=(0, None, None, None, None, None, None, None))


def kernel(inputs, w1, b1, w2, b2, gate_w, gate_b, sew):
    B = inputs.shape[0]
    xs = np.asarray(inputs).reshape(M, B // M, T, D)
    out = _pfwd(xs,
                np.asarray(w1), np.asarray(b1),
                np.asarray(w2), np.asarray(b2),
                np.asarray(gate_w), np.asarray(gate_b), np.asarray(sew))
    return np.asarray(out).reshape(B, T, H).astype(np.float32)
